# revision 46
# baseline (speedup 1.0000x reference)
"""Trainium2 Bass kernel: LoRA Conv2d mixture-of-experts (moe_routing).

Math reformulation
------------------
reference:  out = sum_e probs[e] * conv_{3x1}(conv_{1x3}(x, w_in[e]), w_out[e])

Both convs are linear, and each expert's rank channels are independent, so
stacking all experts on the rank axis turns the whole MoE into TWO dense
convolutions with no per-expert work at all:

    h   = conv_{1x3}(x, W1)       W1[(e,r), ci, kx]   = w_in[e,r,ci,0,kx]
    out = conv_{3x1}(h, W2)       W2[co, (e,r), ky]   = probs[e]*w_out[e,co,r,ky,0]

(the probability weighting folds into the second conv's weights).
E*R = 64 intermediate channels; identical FLOPs to the reference, tiny
intermediate that never leaves SBUF.

Mapping to the PE array (per chunk of 8 image rows = 512 pixels = one
PSUM bank):
  conv1: out[64->dup 128, 512] += lhsT[ci=128, 128].T @ x[ci=128, (8,64)]
         3 kx taps x 2 ci-blocks = 6 accumulating matmuls. x is padded on
         the HOST with a zero column on each side of W, so every tap reads
         a full even-width [8 rows, 64] window at column offset kx (the
         fp32r ISA requires even inner counts and contiguous aligned dsts).
  conv1's lhsT duplicates the 64 h-channels across all 128 array columns;
  the two product copies are drained to SBUF one image-row apart, so conv2
  contracts ky=0 and ky=1 in a single K=128 matmul (rhs partitions 0-63
  hold h[y-1], partitions 64-127 hold h[y]):
  conv2: out[co=128, 512] = lhsT01[128, 128].T @ hdup[:, y*64:]      (ky=0,1)
                          + lhsT2  [64, 128].T @ h-half               (ky=2)
  and the two K=64 ky=2 taps (one per co-block) execute CONCURRENTLY in
  disjoint PE row groups via tile_position. Image top/bottom come from
  zeroed pad rows in the SBUF h buffer.

Synchronization/perf notes (this toolchain lowers one semaphore wait per
engine instruction; bacc's generate_event_semaphores legalizes the rest):
  - all SBUF/PSUM tiles are static, round-robined in python, to avoid pool
    release nodes joining multi-engine waits onto fp32r matmuls;
  - PSUM drains are split across DVE and ACT so neither engine paces the
    pipeline; warm-up matmuls ramp the PE HAM clock during the first DMA;
  - the DMA stream is serialized at the per-core HBM rate (~350 GB/s), so
    x ships in kb-interleaved pieces sized to what the next chunks need,
    and staged output ships in 0.5 MB pieces as soon as chunks drain.

Sharding: data-parallel over batch, B=16 -> 2 images per NeuronCore x 8.
All matmuls run as float32r (TF32: full-rate fp32 path, N>=256); inputs
are pre-rounded to the TF32 grid on the host. End-to-end relative error
vs the fp32 reference is ~3.6e-4.
"""

import numpy as np

from concourse import bacc, bass, mybir, tile
from concourse import bass_utils

B, CIN, H, W = 16, 256, 64, 64
E, R, COUT = 8, 8, 256
ER = E * R
NCORES = 8
BS = B // NCORES           # images per core
HW = H * W                 # 4096 pixels
WP = W + 2                 # host-padded row width (zero col at each edge)
HWP = H * WP               # padded pixels per channel
CHUNK = 512                # pixels per PSUM bank (8 image rows)
NCHUNK = HW // CHUNK       # 8
ROWS = CHUNK // W          # 8 image rows per chunk

F32 = mybir.dt.float32


def _body(nc, tc, x_d, w_d, o_d):
    F32R = mybir.dt.float32r
    with tc.tile_pool(name="wp", bufs=1) as wpool, \
         tc.tile_pool(name="xp", bufs=1) as xpool, \
         tc.tile_pool(name="hpool", bufs=1) as hpool, \
         tc.tile_pool(name="op", bufs=1) as opool, \
         tc.tile_pool(name="ps", bufs=1, space="PSUM") as pspool:
        # All tiles below are STATIC (one tile per tag, reused round-robin
        # by the python code, never returned to a pool). Pool slot-release
        # nodes would join a PE wait + a DVE wait onto the first matmul of
        # a reused slot, but an fp32r matmul can carry only ONE semaphore
        # wait (the LDWEIGHTS half of the fused instruction has a single
        # free sync slot). With static tiles the PE-side WAW is implied by
        # program order and only the single DVE drain wait remains.
        # weight table ships compact (393 KB): six [128,64] conv1 half
        # tiles (cols 0-383), conv2 ky01 packed tiles (384-639), and the
        # two K=64 ky2 tiles row-packed into one [128,128] tile (640-767).
        # conv1's lhsT needs the 64 h-channels duplicated across the full
        # 128 array columns, so it is expanded on-chip into wdup by twelve
        # tiny DVE copies instead of shipping the duplicate bytes.
        wsm = wpool.tile([128, 768], F32R, tag="wsm", name="wsm")
        wdup = wpool.tile([128, 768], F32R, tag="wdup", name="wdup")

        def wtile(i):
            return wdup[:, i * 128:(i + 1) * 128]

        xts = [xpool.tile([128, 2 * HWP], F32R, tag=f"x{b}", name=f"x{b}")
               for b in range(BS)]
        hps = [hpool.tile([128, (H + 2) * W], F32R, tag=f"h{b}", name=f"h{b}")
               for b in range(BS)]
        osts = [opool.tile([128, 2 * HW], F32, tag=f"ost{b}", name=f"ost{b}")
                for b in range(BS)]
        pss = [pspool.tile([128, CHUNK], F32, tag=f"ps{i}", name=f"ps{i}") for i in range(7)]
        scratch = pspool.tile([128, CHUNK], F32, tag="sc", name="scratch")
        n_ps = [0]

        def next_ps():
            t = pss[n_ps[0] % len(pss)]
            n_ps[0] += 1
            return t

        # Every conv1 chunk needs BOTH kb blocks, so image 0's x ships as
        # interleaved kb0/kb1 pieces with small leading pieces (rows 0-7,
        # 8-15, 16-31, 32-63): the DMA stream is serialized at the HBM
        # rate and chunk c must not wait for bytes it doesn't need yet.
        nc.sync.dma_start(out=wsm, in_=w_d)
        for i in range(6):
            for half in range(2):
                nc.vector.tensor_copy(
                    out=wdup[:, i * 128 + half * 64:i * 128 + (half + 1) * 64],
                    in_=wsm[:, i * 64:(i + 1) * 64])
        srcs = [x_d[0, kb * 128:(kb + 1) * 128].rearrange("c h w -> c (h w)")
                for kb in range(2)]
        bounds = [0, 8 * WP, 16 * WP, 32 * WP, HWP]
        for p in range(4):
            lo, hi = bounds[p], bounds[p + 1]
            for kb in range(2):
                nc.sync.dma_start(
                    out=xts[0][:, kb * HWP + lo:kb * HWP + hi],
                    in_=srcs[kb][:, lo:hi])
        for b in range(1, BS):
            bsrcs = [x_d[b, kb * 128:(kb + 1) * 128].rearrange("c h w -> c (h w)")
                     for kb in range(2)]
            hh = HWP // 2
            for p in range(2):
                for kb in range(2):
                    nc.sync.dma_start(
                        out=xts[b][:, kb * HWP + p * hh:kb * HWP + (p + 1) * hh],
                        in_=bsrcs[kb][:, p * hh:(p + 1) * hh])

        # padded h layout: addr a = row_idx*W + w, row_idx 0..65
        #   partitions  0- 63: h[row_idx-1]  (rows stored at 1..64)
        #   partitions 64-127: h[row_idx]    (rows stored at 0..63)
        # pad rows are zeroed once; data copies never touch them.
        for hp in hps:
            nc.vector.memset(hp[0:64, 0:W].bitcast(F32), 0.0)
            nc.vector.memset(hp[0:64, (H + 1) * W:(H + 2) * W].bitcast(F32), 0.0)
            nc.vector.memset(hp[64:128, H * W:(H + 2) * W].bitcast(F32), 0.0)

        # HAM warm-up: dummy matmuls on a never-written SBUF tile (garbage
        # values, result never read) so PE starts ramping at t=0 instead of
        # waiting for the weight-table DMA.
        warm = wpool.tile([128, CHUNK], F32R, tag="warm", name="warm")
        nc.vector.memset(warm.bitcast(F32), 0.0)
        for i in range(8):
            nc.tensor.matmul(scratch, warm[:, 0:128], warm,
                             start=True, stop=True, skip_group_check=True)

        for b in range(BS):
            hp = hps[b]

            for c in range(NCHUNK):
                p0 = c * CHUNK
                r0 = c * ROWS
                ps = next_ps()
                xrs = [xts[b][:, kb * HWP:(kb + 1) * HWP] for kb in range(2)]
                x3s = [xr.rearrange("k (h w) -> k h w", w=WP) for xr in xrs]
                # x rows are host-padded with a zero column at each edge, so
                # every tap kx reads a full even-width [8, 64] window at
                # column offset kx and writes the full contiguous 512 dst
                # (fp32r ISA: src inner count even, dst contiguous+aligned).
                for i, (kb, kx) in enumerate(
                        [(0, 1), (1, 1), (0, 2), (1, 2), (0, 0), (1, 0)]):
                    nc.tensor.matmul(ps, wtile(kb * 3 + kx),
                                     x3s[kb][:, r0:r0 + ROWS, kx:kx + W],
                                     start=(i == 0), stop=(i == 5))
                # h rows r0..r0+7: lower half at (r+1)*W, upper half at r*W.
                # One drain on DVE, one on ACT so neither engine gates the
                # conv1 phase (each copy ~0.9us vs PE's ~1.5us per chunk).
                nc.vector.tensor_copy(out=hp[0:64, p0 + W:p0 + W + CHUNK],
                                      in_=ps[0:64, :])
                nc.scalar.copy(out=hp[64:128, p0:p0 + CHUNK],
                               in_=ps[64:128, :])

            hr = hp
            ost = osts[b]
            for c in range(NCHUNK):
                p0 = c * CHUNK
                ps2s = [next_ps() for _ in range(2)]
                for mb in range(2):
                    nc.tensor.matmul(ps2s[mb], wsm[:, 384 + mb * 128:512 + mb * 128],
                                     hr[:, p0:p0 + CHUNK],
                                     start=True, stop=False)
                # the two K=64 ky=2 taps run CONCURRENTLY in disjoint PE row
                # groups: mb0 in rows 0-63 reads the lower h half (h[y+1] at
                # (y+2)*W), mb1 in rows 64-127 reads the upper h half
                # (h[y+1] at (y+1)*W); mb1's lhsT lives in partitions 64-127
                # of its weight tile.
                nc.tensor.matmul(ps2s[0], wsm[0:64, 640:768],
                                 hr[0:64, p0 + 2 * W:p0 + 2 * W + CHUNK],
                                 start=False, stop=True)
                nc.tensor.matmul(ps2s[1], wsm[64:128, 640:768],
                                 hr[64:128, p0 + W:p0 + W + CHUNK],
                                 start=False, stop=True, tile_position=(64, 0))
                for mb in range(2):
                    # one conv2 drain per chunk on each engine; bacc's
                    # generate_event_semaphores legalizes the multi-wait
                    # instructions this creates.
                    o0 = mb * HW + p0
                    if mb == 0:
                        nc.vector.tensor_copy(out=ost[:, o0:o0 + CHUNK],
                                              in_=ps2s[mb])
                    else:
                        nc.scalar.copy(out=ost[:, o0:o0 + CHUNK], in_=ps2s[mb])
                # ship staged output in 0.5 MB pieces as soon as chunks
                # drain: the serialized DMA stream must never starve while
                # compute is still producing
                ship = {1: (0, 2), 3: (2, 4), 5: (4, 6), 7: (6, 8)}.get(c)
                if ship is not None:
                    c0, c1 = ship
                    for mb in range(2):
                        odst = o_d[b, mb * 128:(mb + 1) * 128].rearrange(
                            "c h w -> c (h w)")
                        nc.sync.dma_start(
                            out=odst[:, c0 * CHUNK:c1 * CHUNK],
                            in_=ost[:, mb * HW + c0 * CHUNK:
                                    mb * HW + c1 * CHUNK])


def _build():
    nc = bacc.Bacc("TRN2", target_bir_lowering=False, debug=False)
    x_d = nc.dram_tensor("x", [BS, CIN, H, WP], mybir.dt.float32r, kind="ExternalInput").ap()
    w_d = nc.dram_tensor("wtab", [128, 768], mybir.dt.float32r, kind="ExternalInput").ap()
    o_d = nc.dram_tensor("out", [BS, COUT, H, W], F32, kind="ExternalOutput").ap()
    with tile.TileContext(nc) as tc:
        _body(nc, tc, x_d, w_d, o_d)
    nc.compile()
    return nc


def _prep_weights(probs, weight_in, weight_out):
    """Host-side lhsT tables, one [128, 1280] f32 array (10 tiles of 128 cols).

    tiles 0-5: conv1 lhsT for (kb, kx) pairs, [ci_local, m] with the 64
               h-channels duplicated to m and m+64.
    tiles 6-7: conv2 ky=0 (rows 0-63) + ky=1 (rows 64-127), per co-block.
    tiles 8-9: conv2 ky=2 (rows 0-63, rest zero), per co-block.
    """
    w1 = np.ascontiguousarray(weight_in[:, :, :, 0, :]).reshape(ER, CIN, 3)
    w2 = weight_out[:, :, :, :, 0] * probs[:, None, None, None]   # [e,co,r,ky]
    w2 = np.ascontiguousarray(w2.transpose(3, 0, 2, 1)).reshape(3, ER, COUT)
    wtab = np.zeros((128, 768), np.float32)
    for kb in range(2):
        for kx in range(3):
            i = kb * 3 + kx
            wtab[:, i * 64:(i + 1) * 64] = w1[:, kb * 128:(kb + 1) * 128, kx].T
    for mb in range(2):
        cs = slice(mb * 128, (mb + 1) * 128)
        wtab[0:64, 384 + mb * 128:512 + mb * 128] = w2[0][:, cs]
        wtab[64:128, 384 + mb * 128:512 + mb * 128] = w2[1][:, cs]
    # ky=2 lhsT row-packed: mb0 in partitions 0-63, mb1 in 64-127 (the
    # kernel runs the two K=64 taps concurrently in disjoint PE row groups)
    wtab[0:64, 640:768] = w2[2][:, 0:128]
    wtab[64:128, 640:768] = w2[2][:, 128:256]
    return wtab


def _tf32_round(a):
    """Round-to-nearest-even to the 10-bit-mantissa fp32r (TF32) grid."""
    b = np.ascontiguousarray(a, dtype=np.float32).view(np.uint32)
    lsb = (b >> np.uint32(13)) & np.uint32(1)
    b = (b + np.uint32(0x0FFF) + lsb) & np.uint32(0xFFFFE000)
    return b.view(np.float32)


_NC_CACHE = []


def _run(x, probs, weight_in, weight_out, trace=False):
    x = _tf32_round(np.asarray(x, dtype=np.float32))
    # pad a zero column on each side of W so the kernel's shifted conv1
    # reads are full-width (fp32r ISA requires even inner counts)
    xp = np.zeros((B, CIN, H, WP), np.float32)
    xp[:, :, :, 1:W + 1] = x
    x = xp
    wtab = _prep_weights(np.asarray(probs, dtype=np.float32),
                         np.asarray(weight_in, dtype=np.float32),
                         np.asarray(weight_out, dtype=np.float32))
    wtab = _tf32_round(wtab)
    if not _NC_CACHE:
        _NC_CACHE.append(_build())
    nc = _NC_CACHE[0]
    in_maps = [{"x": np.ascontiguousarray(x[i * BS:(i + 1) * BS]), "wtab": wtab}
               for i in range(NCORES)]
    res = bass_utils.run_bass_kernel_spmd(
        nc, in_maps, core_ids=list(range(NCORES)), trace=trace)
    out = np.concatenate([r["out"] for r in res.results], axis=0)
    return out, res


def kernel(x, probs, weight_in, weight_out):
    out, _ = _run(x, probs, weight_in, weight_out)
    return out
